# revision 11
# baseline (speedup 1.0000x reference)
"""Baichuan attention prefill on 8 TRN2 NeuronCores.

Tensor-parallel over heads: 5 heads per core. Per core:
  QKV projection (its heads' slice of W_pack) -> attention (causal,
  softmax without max-subtraction) -> AllGather of attention output
  O^T across cores -> o_proj with full contraction producing this
  core's 640 output dims. Host assembles the [1, 2048, 5120] output.

Matmuls run in bf16 (fp32 PSUM accumulation); softmax normalization
in fp32. Q^T/K^T/V stay SBUF-resident in bf16 between projection and
attention; o_proj is chunk-pipelined behind the per-chunk AllGathers.
"""

import numpy as np
import ml_dtypes

import concourse.bacc as bacc
import concourse.mybir as mybir
from concourse.tile import TileContext
from concourse.bass_utils import run_bass_kernel_spmd

HID = 5120
NH = 40
HD = 128
S = 2048
N_CORES = 8
HPC = NH // N_CORES          # 5 heads per core
DPC = HPC * HD               # 640 dims per core
F32 = mybir.dt.float32
BF16 = mybir.dt.bfloat16
SCALE = 1.0 / float(np.sqrt(HD))
MASK_NEG = -30000.0

SB = 1024                    # hid superblock (8 x 128 subtiles)
NSB = HID // SB              # 5 superblocks
NST = SB // 128              # 8 subtiles per superblock
SEQ_HALF = S // 2            # QKV processes seq in halves of 1024
CHUNK = 512                  # attention / o_proj seq chunk
NCHUNK = S // CHUNK          # 4
RG = [list(range(N_CORES))]

_graph_cache = None


def _build_graph():
    nc = bacc.Bacc(name="baichuan_attn")

    xt = nc.declare_dram_parameter("xt", [HID, S], BF16, isOutput=False)
    wqt = nc.declare_dram_parameter("wqt", [HID, DPC], BF16, isOutput=False)
    wkt = nc.declare_dram_parameter("wkt", [HID, DPC], BF16, isOutput=False)
    wvt = nc.declare_dram_parameter("wvt", [HID, DPC], BF16, isOutput=False)
    # wot_t[mt][p][ht*128+c] = o_proj_w[640c_core + 128 mt + c, 128 ht + p]
    wot_t = nc.declare_dram_parameter("wot_t", [HPC, 128, HID], BF16, isOutput=False)
    dmask = nc.declare_dram_parameter("dmask", [4, 128, CHUNK], BF16, isOutput=False)
    ones = nc.declare_dram_parameter("ones", [128, 128], BF16, isOutput=False)
    ident = nc.declare_dram_parameter("ident", [128, 128], BF16, isOutput=False)
    out = nc.declare_dram_parameter("out", [DPC, S], F32, isOutput=True)

    warm_in = nc.dram_tensor("warm_in", [128, 8], BF16)
    warm_out = nc.dram_tensor("warm_out", [1024, 8], BF16, addr_space="Shared")
    ot_b = [nc.dram_tensor(f"ot_b{j}", [DPC, CHUNK], BF16) for j in range(NCHUNK)]
    og = [
        nc.dram_tensor(f"og{j}", [HID, CHUNK], BF16, addr_space="Shared")
        for j in range(NCHUNK)
    ]

    with TileContext(nc) as tc:
        nc.gpsimd.collective_compute(
            "AllGather",
            mybir.AluOpType.bypass,
            replica_groups=RG,
            ins=[warm_in.ap().opt()],
            outs=[warm_out.ap().opt()],
        )
        with tc.tile_pool(name="qkv_sb", bufs=1) as sbp:
            q_sb = sbp.tile([128, HPC, S], BF16, name="q_sb")
            k_sb = sbp.tile([128, HPC, S], BF16, name="k_sb")
            v_sb = sbp.tile([128, S // 128, DPC], BF16, name="v_sb")
            _qkv_phase(nc, tc, xt, wqt, wkt, wvt, q_sb, k_sb, v_sb)
            _attn_oproj_phase(
                nc, tc, dmask, ones, ident, q_sb, k_sb, v_sb, ot_b, og, wot_t, out
            )

    nc.compile()
    return nc


def _qkv_phase(nc, tc, xt, wqt, wkt, wvt, q_sb, k_sb, v_sb):
    with (
        tc.tile_pool(name="qkv_acc", bufs=1) as accp,
        tc.tile_pool(name="qkv_xt", bufs=2) as xtp,
        tc.tile_pool(name="qkv_w", bufs=16) as wp,
        tc.tile_pool(name="qkv_psqk", bufs=4, space="PSUM") as pqk,
        tc.tile_pool(name="qkv_psv", bufs=4, space="PSUM") as pvp,
    ):
        for half in range(2):
            s0 = half * SEQ_HALF
            acc_q = accp.tile([128, HPC, SEQ_HALF], F32, name=f"accq{half}", tag="accq")
            acc_k = accp.tile([128, HPC, SEQ_HALF], F32, name=f"acck{half}", tag="acck")
            acc_v = accp.tile([128, SEQ_HALF // 128, DPC], F32, name=f"accv{half}", tag="accv")
            for b in range(NSB):
                h0 = b * SB
                xtb = xtp.tile([128, NST, SEQ_HALF], BF16, name=f"xtb{half}_{b}", tag="xtb")
                for i in range(NST):
                    nc.sync.dma_start(
                        xtb[:, i, :], xt[h0 + 128 * i : h0 + 128 * (i + 1), s0 : s0 + SEQ_HALF]
                    )
                for wsrc, acc, fin, pname in (
                    (wqt, acc_q, q_sb, "q"),
                    (wkt, acc_k, k_sb, "k"),
                ):
                    wrows = []
                    for i in range(NST):
                        w = wp.tile([128, DPC], BF16, name=f"w{pname}{half}_{b}_{i}", tag="wrow")
                        nc.sync.dma_start(w[:, :], wsrc[h0 + 128 * i : h0 + 128 * (i + 1), :])
                        wrows.append(w)
                    for dt in range(HPC):
                        for jj in range(SEQ_HALF // 512):
                            ps = pqk.tile([128, 512], F32, name=f"ps{pname}", tag="psqk")
                            for i in range(NST):
                                nc.tensor.matmul(
                                    ps[:, :],
                                    lhsT=wrows[i][:, 128 * dt : 128 * (dt + 1)],
                                    rhs=xtb[:, i, 512 * jj : 512 * (jj + 1)],
                                    start=(i == 0),
                                    stop=(i == NST - 1),
                                )
                            acc_sl = acc[:, dt, 512 * jj : 512 * (jj + 1)]
                            if b == 0:
                                nc.scalar.copy(acc_sl, ps[:, :])
                            elif b < NSB - 1:
                                nc.vector.tensor_add(acc_sl, acc_sl, ps[:, :])
                            else:
                                # final add rounds straight into the bf16 store
                                fin_sl = fin[:, dt, s0 + 512 * jj : s0 + 512 * (jj + 1)]
                                nc.vector.tensor_add(fin_sl, acc_sl, ps[:, :])
                # V pass: natural layout [seq, d]
                wrows = []
                for i in range(NST):
                    w = wp.tile([128, DPC], BF16, name=f"wv{half}_{b}_{i}", tag="wrow")
                    nc.sync.dma_start(w[:, :], wvt[h0 + 128 * i : h0 + 128 * (i + 1), :])
                    wrows.append(w)
                for st in range(SEQ_HALF // 128):
                    for nh in range(2):
                        ps = pvp.tile([128, 320], F32, name="psv", tag="psv")
                        for i in range(NST):
                            nc.tensor.matmul(
                                ps[:, :],
                                lhsT=xtb[:, i, 128 * st : 128 * (st + 1)],
                                rhs=wrows[i][:, 320 * nh : 320 * (nh + 1)],
                                start=(i == 0),
                                stop=(i == NST - 1),
                            )
                        acc_sl = acc_v[:, st, 320 * nh : 320 * (nh + 1)]
                        if b == 0:
                            nc.scalar.copy(acc_sl, ps[:, :])
                        elif b < NSB - 1:
                            nc.vector.tensor_add(acc_sl, acc_sl, ps[:, :])
                        else:
                            st_g = (s0 // 128) + st
                            fin_sl = v_sb[:, st_g, 320 * nh : 320 * (nh + 1)]
                            nc.vector.tensor_add(fin_sl, acc_sl, ps[:, :])


def _attn_chunk(nc, j, pools, stores):
    (pp, op_, pss, pso, psl) = pools
    (q_sb, k_sb, v_sb, mask_sb, ones_sb, ident_sb, ot_b, og) = stores
    for h in range(HPC):
        qtile = q_sb[:, h, CHUNK * j : CHUNK * (j + 1)]
        po = pso.tile([128, CHUNK], F32, name=f"po{j}_{h}", tag="po")
        pl = psl.tile([128, CHUNK], F32, name=f"pl{j}_{h}", tag="pl")
        nk = 4 * (j + 1)

        # software-pipelined: emit S-matmul of tile kt before the
        # exp/PV/L of tile kt-1 so PE stays busy during exp.
        pending = None
        for kt in range(nk):
            ps = pss.tile([128, CHUNK], F32, name=f"ps{j}_{h}_{kt}", tag="ps")
            diag = kt >= 4 * j
            if diag:
                # write causal mask into PSUM on PE itself (identity matmul),
                # then accumulate the scores onto it in the same group
                nc.tensor.matmul(
                    ps[:, :], lhsT=ident_sb[:, :], rhs=mask_sb[:, kt - 4 * j, :],
                    start=True, stop=False,
                )
            nc.tensor.matmul(
                ps[:, :],
                lhsT=k_sb[:, h, 128 * kt : 128 * (kt + 1)],
                rhs=qtile,
                start=not diag,
                stop=True,
            )
            if pending is not None:
                _attn_tail(nc, pp, v_sb, po, pl, ones_sb, j, h, pending, nk)
            pending = (ps, kt)
        _attn_tail(nc, pp, v_sb, po, pl, ones_sb, j, h, pending, nk)

        linv = op_.tile([128, CHUNK], F32, name=f"linv{j}_{h}", tag="linv")
        nc.vector.reciprocal(linv[:, :], pl[:, :])
        ot = op_.tile([128, CHUNK], BF16, name=f"ot{j}_{h}", tag="ot")
        nc.vector.tensor_mul(ot[:, :], po[:, :], linv[:, :])
        nc.gpsimd.dma_start(ot_b[j][128 * h : 128 * (h + 1), :], ot[:, :])
    nc.gpsimd.collective_compute(
        "AllGather",
        mybir.AluOpType.bypass,
        replica_groups=RG,
        ins=[ot_b[j].ap().opt()],
        outs=[og[j].ap().opt()],
    )


def _attn_tail(nc, pp, v_sb, po, pl, ones_sb, j, h, pending, nk):
    ps, kt = pending
    ptile = pp.tile([128, CHUNK], BF16, name=f"pt{j}_{h}_{kt}", tag="pt")
    nc.scalar.activation(
        ptile[:, :], ps[:, :], mybir.ActivationFunctionType.Exp,
        bias=0.0, scale=SCALE,
    )
    nc.tensor.matmul(
        po[:, :], lhsT=v_sb[:, kt, 128 * h : 128 * (h + 1)], rhs=ptile[:, :],
        start=(kt == 0), stop=(kt == nk - 1),
    )
    nc.tensor.matmul(
        pl[:, :], lhsT=ones_sb[:, :], rhs=ptile[:, :],
        start=(kt == 0), stop=(kt == nk - 1),
    )


def _oproj_chunk(nc, jj, ogp, wp, psp, yp, og, wot_t, out):
    """o_proj for one seq chunk: full 5120 contraction per PSUM group."""
    halves = []
    for piece in range(2):
        ogt = ogp.tile([128, NH // 2, CHUNK], BF16, name=f"ogt{jj}_{piece}", tag="ogt")
        for hh in range(NH // 2):
            ht = piece * (NH // 2) + hh
            nc.sync.dma_start(
                ogt[:, hh, :], og[jj][128 * ht : 128 * (ht + 1), :]
            )
        halves.append(ogt)
    for mt in range(HPC):
        wcols = []
        for piece in range(2):
            wcol = wp.tile(
                [128, NH // 2, 128], BF16, name=f"wo{jj}_{mt}_{piece}", tag="wcol"
            )
            nc.sync.dma_start(
                wcol[:, :, :],
                wot_t[mt, :, piece * (HID // 2) : (piece + 1) * (HID // 2)].rearrange(
                    "p (a b) -> p a b", a=NH // 2
                ),
            )
            wcols.append(wcol)
        ps = psp.tile([128, CHUNK], F32, name=f"py{jj}_{mt}", tag="py")
        for ht in range(NH):
            piece, hh = divmod(ht, NH // 2)
            nc.tensor.matmul(
                ps[:, :],
                lhsT=wcols[piece][:, hh, :],
                rhs=halves[piece][:, hh, :],
                start=(ht == 0),
                stop=(ht == NH - 1),
            )
        ysb = yp.tile([128, CHUNK], F32, name=f"y{jj}_{mt}", tag="y")
        nc.scalar.copy(ysb[:, :], ps[:, :])
        nc.gpsimd.dma_start(
            out[128 * mt : 128 * (mt + 1), CHUNK * jj : CHUNK * (jj + 1)], ysb[:, :]
        )


def _attn_oproj_phase(nc, tc, dmask, ones, ident, q_sb, k_sb, v_sb, ot_b, og, wot_t, out):
    with (
        tc.tile_pool(name="at_const", bufs=1) as cstp,
        tc.tile_pool(name="at_p", bufs=6) as pp,
        tc.tile_pool(name="at_o", bufs=3) as op_,
        tc.tile_pool(name="op_og", bufs=4) as ogp,
        tc.tile_pool(name="op_w", bufs=4) as wp,
        tc.tile_pool(name="op_y", bufs=3) as yp,
        tc.tile_pool(name="at_pss", bufs=3, space="PSUM") as pss,
        tc.tile_pool(name="at_pso", bufs=2, space="PSUM") as pso,
        tc.tile_pool(name="at_psl", bufs=1, space="PSUM") as psl,
        tc.tile_pool(name="op_ps", bufs=2, space="PSUM") as psp,
    ):
        mask_sb = cstp.tile([128, 4, CHUNK], BF16, name="mask_sb")
        for t in range(4):
            nc.sync.dma_start(mask_sb[:, t, :], dmask[t, :, :])
        ones_sb = cstp.tile([128, 128], BF16, name="ones_sb")
        nc.sync.dma_start(ones_sb[:, :], ones[:, :])
        ident_sb = cstp.tile([128, 128], BF16, name="ident_sb")
        nc.sync.dma_start(ident_sb[:, :], ident[:, :])

        pools = (pp, op_, pss, pso, psl)
        stores = (q_sb, k_sb, v_sb, mask_sb, ones_sb, ident_sb, ot_b, og)
        # chunk-level pipeline: o_proj of chunk j-1 is emitted after
        # attention+AG of chunk j, so PE runs attention while AG(j-1)
        # completes, then o_proj(j-1) while AG(j) flies.
        for j in range(NCHUNK):
            _attn_chunk(nc, j, pools, stores)
        for jj in range(NCHUNK):
            _oproj_chunk(nc, jj, ogp, wp, psp, yp, og, wot_t, out)


def _to_bf16(a):
    return np.asarray(a, dtype=np.float32).astype(ml_dtypes.bfloat16)


def _prep_inputs(hidden_states, W_pack_w, o_proj_w):
    xt = _to_bf16(np.ascontiguousarray(hidden_states.reshape(S, HID).T))
    dmask = np.zeros((4, 128, CHUNK), dtype=np.float32)
    for t in range(4):
        for p in range(128):
            k = 128 * t + p
            dmask[t, p, : min(k, CHUNK)] = MASK_NEG
    # dmask[t, p, q] must be MASK_NEG where q < 128 t + p (future key), 0 else
    dmask = dmask.astype(ml_dtypes.bfloat16)
    ones = np.ones((128, 128), dtype=ml_dtypes.bfloat16)
    ident = np.eye(128, dtype=ml_dtypes.bfloat16)
    in_maps = []
    for c in range(N_CORES):
        r0 = DPC * c
        # wot_t[mt][p][ht*128+c2] = o_proj_w[r0 + 128 mt + c2, 128 ht + p]
        woc = o_proj_w[r0 : r0 + DPC, :]          # [640 out, 5120 in]
        wot_t = np.ascontiguousarray(
            woc.reshape(HPC, 128, NH, 128).transpose(0, 3, 2, 1).reshape(HPC, 128, HID)
        )
        in_maps.append(
            {
                "xt": xt,
                "wqt": _to_bf16(np.ascontiguousarray(W_pack_w[r0 : r0 + DPC, :].T)),
                "wkt": _to_bf16(np.ascontiguousarray(W_pack_w[HID + r0 : HID + r0 + DPC, :].T)),
                "wvt": _to_bf16(np.ascontiguousarray(W_pack_w[2 * HID + r0 : 2 * HID + r0 + DPC, :].T)),
                "wot_t": _to_bf16(wot_t),
                "dmask": dmask,
                "ones": ones,
                "ident": ident,
            }
        )
    return in_maps


def run(hidden_states, W_pack_w, o_proj_w, trace=False):
    global _graph_cache
    if _graph_cache is None:
        _graph_cache = _build_graph()
    nc = _graph_cache
    in_maps = _prep_inputs(hidden_states, W_pack_w, o_proj_w)
    res = run_bass_kernel_spmd(nc, in_maps, list(range(N_CORES)), trace=trace)
    y = np.concatenate([res.results[c]["out"].T for c in range(N_CORES)], axis=1)
    return y.reshape(1, S, HID), res


def kernel(
    hidden_states,
    W_pack_w,
    o_proj_w,
    k_cache=None,
    v_cache=None,
    input_pos=None,
    attention_mask=None,
    **_unused,
):
    hidden_states = np.asarray(hidden_states, dtype=np.float32)
    W_pack_w = np.asarray(W_pack_w, dtype=np.float32)
    o_proj_w = np.asarray(o_proj_w, dtype=np.float32)
    y, _ = run(hidden_states, W_pack_w, o_proj_w, trace=False)
    return y


# revision 12
# speedup vs baseline: 1.0363x; 1.0363x over previous
"""Baichuan attention prefill on 8 TRN2 NeuronCores.

Tensor-parallel over heads: 5 heads per core. Per core:
  QKV projection (its heads' slice of W_pack) -> attention (causal,
  softmax without max-subtraction) -> AllGather of attention output
  O^T across cores -> o_proj with full contraction producing this
  core's 640 output dims. Host assembles the [1, 2048, 5120] output.

Matmuls run in bf16 (fp32 PSUM accumulation); softmax normalization
in fp32. Q^T/K^T/V stay SBUF-resident in bf16 between projection and
attention; o_proj is chunk-pipelined behind the per-chunk AllGathers.
"""

import numpy as np
import ml_dtypes

import concourse.bacc as bacc
import concourse.mybir as mybir
from concourse.tile import TileContext
from concourse.bass_utils import run_bass_kernel_spmd

HID = 5120
NH = 40
HD = 128
S = 2048
N_CORES = 8
HPC = NH // N_CORES          # 5 heads per core
DPC = HPC * HD               # 640 dims per core
F32 = mybir.dt.float32
BF16 = mybir.dt.bfloat16
SCALE = 1.0 / float(np.sqrt(HD))
MASK_NEG = -30000.0

SB = 1024                    # hid superblock (8 x 128 subtiles)
NSB = HID // SB              # 5 superblocks
NST = SB // 128              # 8 subtiles per superblock
SEQ_HALF = S // 2            # QKV processes seq in halves of 1024
CHUNK = 512                  # attention / o_proj seq chunk
NCHUNK = S // CHUNK          # 4
RG = [list(range(N_CORES))]

_graph_cache = None


def _build_graph():
    nc = bacc.Bacc(name="baichuan_attn")

    xt = nc.declare_dram_parameter("xt", [HID, S], BF16, isOutput=False)
    wqt = nc.declare_dram_parameter("wqt", [HID, DPC], BF16, isOutput=False)
    wkt = nc.declare_dram_parameter("wkt", [HID, DPC], BF16, isOutput=False)
    wvt = nc.declare_dram_parameter("wvt", [HID, DPC], BF16, isOutput=False)
    # wot_t[mt][p][ht*128+c] = o_proj_w[640c_core + 128 mt + c, 128 ht + p]
    wot_t = nc.declare_dram_parameter("wot_t", [HPC, 128, HID], BF16, isOutput=False)
    dmask = nc.declare_dram_parameter("dmask", [4, 128, CHUNK], BF16, isOutput=False)
    ones = nc.declare_dram_parameter("ones", [128, 128], BF16, isOutput=False)
    ident = nc.declare_dram_parameter("ident", [128, 128], BF16, isOutput=False)
    out = nc.declare_dram_parameter("out", [DPC, S], F32, isOutput=True)

    warm_in = nc.dram_tensor("warm_in", [128, 8], BF16)
    warm_out = nc.dram_tensor("warm_out", [1024, 8], BF16, addr_space="Shared")
    ot_b = [nc.dram_tensor(f"ot_b{j}", [DPC, CHUNK], BF16) for j in range(NCHUNK)]
    og = [
        nc.dram_tensor(f"og{j}", [HID, CHUNK], BF16, addr_space="Shared")
        for j in range(NCHUNK)
    ]

    with TileContext(nc) as tc:
        nc.gpsimd.collective_compute(
            "AllGather",
            mybir.AluOpType.bypass,
            replica_groups=RG,
            ins=[warm_in.ap().opt()],
            outs=[warm_out.ap().opt()],
        )
        with tc.tile_pool(name="qkv_sb", bufs=1) as sbp:
            q_sb = sbp.tile([128, HPC, S], BF16, name="q_sb")
            k_sb = sbp.tile([128, HPC, S], BF16, name="k_sb")
            v_sb = sbp.tile([128, S // 128, DPC], BF16, name="v_sb")
            _qkv_phase(nc, tc, xt, wqt, wkt, wvt, q_sb, k_sb, v_sb)
            _attn_oproj_phase(
                nc, tc, dmask, ones, ident, q_sb, k_sb, v_sb, ot_b, og, wot_t, out
            )
        _oproj_phase(nc, tc, og, wot_t, out)

    nc.compile()
    return nc


def _qkv_phase(nc, tc, xt, wqt, wkt, wvt, q_sb, k_sb, v_sb):
    with (
        tc.tile_pool(name="qkv_acc", bufs=1) as accp,
        tc.tile_pool(name="qkv_xt", bufs=2) as xtp,
        tc.tile_pool(name="qkv_w", bufs=24) as wp,
        tc.tile_pool(name="qkv_psqk", bufs=4, space="PSUM") as pqk,
        tc.tile_pool(name="qkv_psv", bufs=4, space="PSUM") as pvp,
    ):
        for half in range(2):
            s0 = half * SEQ_HALF
            acc_q = accp.tile([128, HPC, SEQ_HALF], F32, name=f"accq{half}", tag="accq")
            acc_k = accp.tile([128, HPC, SEQ_HALF], F32, name=f"acck{half}", tag="acck")
            acc_v = accp.tile([128, SEQ_HALF // 128, DPC], F32, name=f"accv{half}", tag="accv")
            for b in range(NSB):
                h0 = b * SB
                xtb = xtp.tile([128, NST, SEQ_HALF], BF16, name=f"xtb{half}_{b}", tag="xtb")
                for i in range(NST):
                    nc.sync.dma_start(
                        xtb[:, i, :], xt[h0 + 128 * i : h0 + 128 * (i + 1), s0 : s0 + SEQ_HALF]
                    )
                for wsrc, acc, fin, pname in (
                    (wqt, acc_q, q_sb, "q"),
                    (wkt, acc_k, k_sb, "k"),
                ):
                    wrows = []
                    for i in range(NST):
                        w = wp.tile([128, DPC], BF16, name=f"w{pname}{half}_{b}_{i}", tag="wrow")
                        nc.sync.dma_start(w[:, :], wsrc[h0 + 128 * i : h0 + 128 * (i + 1), :])
                        wrows.append(w)
                    for dt in range(HPC):
                        for jj in range(SEQ_HALF // 512):
                            ps = pqk.tile([128, 512], F32, name=f"ps{pname}", tag="psqk")
                            for i in range(NST):
                                nc.tensor.matmul(
                                    ps[:, :],
                                    lhsT=wrows[i][:, 128 * dt : 128 * (dt + 1)],
                                    rhs=xtb[:, i, 512 * jj : 512 * (jj + 1)],
                                    start=(i == 0),
                                    stop=(i == NST - 1),
                                )
                            acc_sl = acc[:, dt, 512 * jj : 512 * (jj + 1)]
                            if b == 0:
                                nc.scalar.copy(acc_sl, ps[:, :])
                            elif b < NSB - 1:
                                nc.vector.tensor_add(acc_sl, acc_sl, ps[:, :])
                            else:
                                # final add rounds straight into the bf16 store
                                fin_sl = fin[:, dt, s0 + 512 * jj : s0 + 512 * (jj + 1)]
                                nc.vector.tensor_add(fin_sl, acc_sl, ps[:, :])
                # V pass: natural layout [seq, d]
                wrows = []
                for i in range(NST):
                    w = wp.tile([128, DPC], BF16, name=f"wv{half}_{b}_{i}", tag="wrow")
                    nc.sync.dma_start(w[:, :], wvt[h0 + 128 * i : h0 + 128 * (i + 1), :])
                    wrows.append(w)
                for st in range(SEQ_HALF // 128):
                    for nh in range(2):
                        ps = pvp.tile([128, 320], F32, name="psv", tag="psv")
                        for i in range(NST):
                            nc.tensor.matmul(
                                ps[:, :],
                                lhsT=xtb[:, i, 128 * st : 128 * (st + 1)],
                                rhs=wrows[i][:, 320 * nh : 320 * (nh + 1)],
                                start=(i == 0),
                                stop=(i == NST - 1),
                            )
                        acc_sl = acc_v[:, st, 320 * nh : 320 * (nh + 1)]
                        if b == 0:
                            nc.scalar.copy(acc_sl, ps[:, :])
                        elif b < NSB - 1:
                            nc.vector.tensor_add(acc_sl, acc_sl, ps[:, :])
                        else:
                            st_g = (s0 // 128) + st
                            fin_sl = v_sb[:, st_g, 320 * nh : 320 * (nh + 1)]
                            nc.vector.tensor_add(fin_sl, acc_sl, ps[:, :])


def _attn_chunk(nc, j, pools, stores):
    (pp, op_, pss, pso, psl) = pools
    (q_sb, k_sb, v_sb, mask_sb, ones_sb, ident_sb, ot_b, og) = stores
    for h in range(HPC):
        qtile = q_sb[:, h, CHUNK * j : CHUNK * (j + 1)]
        po = pso.tile([128, CHUNK], F32, name=f"po{j}_{h}", tag="po")
        pl = psl.tile([128, CHUNK], F32, name=f"pl{j}_{h}", tag="pl")
        nk = 4 * (j + 1)

        # software-pipelined: emit S-matmul of tile kt before the
        # exp/PV/L of tile kt-1 so PE stays busy during exp.
        pending = None
        for kt in range(nk):
            ps = pss.tile([128, CHUNK], F32, name=f"ps{j}_{h}_{kt}", tag="ps")
            diag = kt >= 4 * j
            if diag:
                # write causal mask into PSUM on PE itself (identity matmul),
                # then accumulate the scores onto it in the same group
                nc.tensor.matmul(
                    ps[:, :], lhsT=ident_sb[:, :], rhs=mask_sb[:, kt - 4 * j, :],
                    start=True, stop=False,
                )
            nc.tensor.matmul(
                ps[:, :],
                lhsT=k_sb[:, h, 128 * kt : 128 * (kt + 1)],
                rhs=qtile,
                start=not diag,
                stop=True,
            )
            if pending is not None:
                _attn_tail(nc, pp, v_sb, po, pl, ones_sb, j, h, pending, nk)
            pending = (ps, kt)
        _attn_tail(nc, pp, v_sb, po, pl, ones_sb, j, h, pending, nk)

        linv = op_.tile([128, CHUNK], F32, name=f"linv{j}_{h}", tag="linv")
        nc.vector.reciprocal(linv[:, :], pl[:, :])
        ot = op_.tile([128, CHUNK], BF16, name=f"ot{j}_{h}", tag="ot")
        nc.vector.tensor_mul(ot[:, :], po[:, :], linv[:, :])
        nc.gpsimd.dma_start(ot_b[j][128 * h : 128 * (h + 1), :], ot[:, :])
    nc.gpsimd.collective_compute(
        "AllGather",
        mybir.AluOpType.bypass,
        replica_groups=RG,
        ins=[ot_b[j].ap().opt()],
        outs=[og[j].ap().opt()],
    )


def _attn_tail(nc, pp, v_sb, po, pl, ones_sb, j, h, pending, nk):
    ps, kt = pending
    ptile = pp.tile([128, CHUNK], BF16, name=f"pt{j}_{h}_{kt}", tag="pt")
    nc.scalar.activation(
        ptile[:, :], ps[:, :], mybir.ActivationFunctionType.Exp,
        bias=0.0, scale=SCALE,
    )
    nc.tensor.matmul(
        po[:, :], lhsT=v_sb[:, kt, 128 * h : 128 * (h + 1)], rhs=ptile[:, :],
        start=(kt == 0), stop=(kt == nk - 1),
    )
    nc.tensor.matmul(
        pl[:, :], lhsT=ones_sb[:, :], rhs=ptile[:, :],
        start=(kt == 0), stop=(kt == nk - 1),
    )


def _oproj_phase(nc, tc, og, wot_t, out):
    """o_proj: full 5120 contraction per (chunk, out-tile) PSUM group.
    All weight pieces stay SBUF-resident (loaded once)."""
    with (
        tc.tile_pool(name="op_og", bufs=6) as ogp,
        tc.tile_pool(name="op_w", bufs=1) as wp,
        tc.tile_pool(name="op_y", bufs=3) as yp,
        tc.tile_pool(name="op_ps", bufs=3, space="PSUM") as psp,
    ):
        wcols = {}
        for mt in range(HPC):
            for piece in range(2):
                wcol = wp.tile(
                    [128, NH // 2, 128], BF16, name=f"wo{mt}_{piece}", tag=f"wc{mt}_{piece}"
                )
                nc.sync.dma_start(
                    wcol[:, :, :],
                    wot_t[mt, :, piece * (HID // 2) : (piece + 1) * (HID // 2)].rearrange(
                        "p (a b) -> p a b", a=NH // 2
                    ),
                )
                wcols[(mt, piece)] = wcol
        for jj in range(NCHUNK):
            halves = []
            for piece in range(2):
                ogt = ogp.tile([128, NH // 2, CHUNK], BF16, name=f"ogt{jj}_{piece}", tag="ogt")
                for hh in range(NH // 2):
                    ht = piece * (NH // 2) + hh
                    nc.sync.dma_start(
                        ogt[:, hh, :], og[jj][128 * ht : 128 * (ht + 1), :]
                    )
                halves.append(ogt)
            for mt in range(HPC):
                ps = psp.tile([128, CHUNK], F32, name=f"py{jj}_{mt}", tag="py")
                for ht in range(NH):
                    piece, hh = divmod(ht, NH // 2)
                    nc.tensor.matmul(
                        ps[:, :],
                        lhsT=wcols[(mt, piece)][:, hh, :],
                        rhs=halves[piece][:, hh, :],
                        start=(ht == 0),
                        stop=(ht == NH - 1),
                    )
                ysb = yp.tile([128, CHUNK], F32, name=f"y{jj}_{mt}", tag="y")
                nc.scalar.copy(ysb[:, :], ps[:, :])
                nc.gpsimd.dma_start(
                    out[128 * mt : 128 * (mt + 1), CHUNK * jj : CHUNK * (jj + 1)], ysb[:, :]
                )


def _attn_oproj_phase(nc, tc, dmask, ones, ident, q_sb, k_sb, v_sb, ot_b, og, wot_t, out):
    with (
        tc.tile_pool(name="at_const", bufs=1) as cstp,
        tc.tile_pool(name="at_p", bufs=8) as pp,
        tc.tile_pool(name="at_o", bufs=3) as op_,
        tc.tile_pool(name="at_pss", bufs=4, space="PSUM") as pss,
        tc.tile_pool(name="at_pso", bufs=2, space="PSUM") as pso,
        tc.tile_pool(name="at_psl", bufs=2, space="PSUM") as psl,
    ):
        mask_sb = cstp.tile([128, 4, CHUNK], BF16, name="mask_sb")
        for t in range(4):
            nc.sync.dma_start(mask_sb[:, t, :], dmask[t, :, :])
        ones_sb = cstp.tile([128, 128], BF16, name="ones_sb")
        nc.sync.dma_start(ones_sb[:, :], ones[:, :])
        ident_sb = cstp.tile([128, 128], BF16, name="ident_sb")
        nc.sync.dma_start(ident_sb[:, :], ident[:, :])

        pools = (pp, op_, pss, pso, psl)
        stores = (q_sb, k_sb, v_sb, mask_sb, ones_sb, ident_sb, ot_b, og)
        # chunk-level pipeline: o_proj of chunk j-1 is emitted after
        # attention+AG of chunk j, so PE runs attention while AG(j-1)
        # completes, then o_proj(j-1) while AG(j) flies.
        for j in range(NCHUNK):
            _attn_chunk(nc, j, pools, stores)


def _to_bf16(a):
    return np.asarray(a, dtype=np.float32).astype(ml_dtypes.bfloat16)


def _prep_inputs(hidden_states, W_pack_w, o_proj_w):
    xt = _to_bf16(np.ascontiguousarray(hidden_states.reshape(S, HID).T))
    dmask = np.zeros((4, 128, CHUNK), dtype=np.float32)
    for t in range(4):
        for p in range(128):
            k = 128 * t + p
            dmask[t, p, : min(k, CHUNK)] = MASK_NEG
    # dmask[t, p, q] must be MASK_NEG where q < 128 t + p (future key), 0 else
    dmask = dmask.astype(ml_dtypes.bfloat16)
    ones = np.ones((128, 128), dtype=ml_dtypes.bfloat16)
    ident = np.eye(128, dtype=ml_dtypes.bfloat16)
    in_maps = []
    for c in range(N_CORES):
        r0 = DPC * c
        # wot_t[mt][p][ht*128+c2] = o_proj_w[r0 + 128 mt + c2, 128 ht + p]
        woc = o_proj_w[r0 : r0 + DPC, :]          # [640 out, 5120 in]
        wot_t = np.ascontiguousarray(
            woc.reshape(HPC, 128, NH, 128).transpose(0, 3, 2, 1).reshape(HPC, 128, HID)
        )
        in_maps.append(
            {
                "xt": xt,
                "wqt": _to_bf16(np.ascontiguousarray(W_pack_w[r0 : r0 + DPC, :].T)),
                "wkt": _to_bf16(np.ascontiguousarray(W_pack_w[HID + r0 : HID + r0 + DPC, :].T)),
                "wvt": _to_bf16(np.ascontiguousarray(W_pack_w[2 * HID + r0 : 2 * HID + r0 + DPC, :].T)),
                "wot_t": _to_bf16(wot_t),
                "dmask": dmask,
                "ones": ones,
                "ident": ident,
            }
        )
    return in_maps


def run(hidden_states, W_pack_w, o_proj_w, trace=False):
    global _graph_cache
    if _graph_cache is None:
        _graph_cache = _build_graph()
    nc = _graph_cache
    in_maps = _prep_inputs(hidden_states, W_pack_w, o_proj_w)
    res = run_bass_kernel_spmd(nc, in_maps, list(range(N_CORES)), trace=trace)
    y = np.concatenate([res.results[c]["out"].T for c in range(N_CORES)], axis=1)
    return y.reshape(1, S, HID), res


def kernel(
    hidden_states,
    W_pack_w,
    o_proj_w,
    k_cache=None,
    v_cache=None,
    input_pos=None,
    attention_mask=None,
    **_unused,
):
    hidden_states = np.asarray(hidden_states, dtype=np.float32)
    W_pack_w = np.asarray(W_pack_w, dtype=np.float32)
    o_proj_w = np.asarray(o_proj_w, dtype=np.float32)
    y, _ = run(hidden_states, W_pack_w, o_proj_w, trace=False)
    return y
